# revision 21
# baseline (speedup 1.0000x reference)
"""Trainium2 Bass kernel for nn_DictMoEDirect (moe_routing), v3.

Reference computation (shapes hardcoded):
  x = hidden_states.transpose(1,0,2)              # [B,S,H]
  g = mean_s(relu(x@gW1.T + gb1) @ gW2.T + gb2)   # [B,E]
  W1_b = sum_e g[b,e] eW1[e]; b1_b = g[b]@eb1     # per-sample merged MLP
  W2_b = sum_e g[b,e] eW2[e]; b2_b = g[b]@eb2
  y = relu(x@W1_b.T + b1_b) @ W2_b.T + b2_b       # [B,S,H]
  return y.transpose(1,0,2)                       # [S,B,H]

Distribution over 8 NeuronCores (v3):
  - Gate: data-parallel (core b computes g[b]), tiny AllGather of g.
  - FFN: tensor-parallel over DFF (core j owns a 512-wide slice).  Layer-2
    partial products are reduce-scattered in 4 bf16 chunks.
  - NEW in v3: the per-sample weight merges (W_b = sum_e g[b,e] E_e) run as
    single-pass PE matmuls with a block-diagonal g as the moving operand:
      stationary lhsT[(e + 8c), m] = E[e, d=q*16+c, i=k*128+m]   (per k,q)
      moving    rhs[(e + 8c), b*16 + c2] = g[b,e] * delta(c,c2)
      out[m, b*16 + c2] = W_b[d=q*16+c2, i=k*128+m]
    One pass consumes each expert element once and produces ALL 8 samples'
    merged weights at full PE stream rate (128 elem/cycle): 256 x 128-row
    matmuls per layer = ~13.7us, vs ~109us/layer for the diag-matmul trick.
    PSUM->SBUF drains (DVE/ACT alternating) de-interleave (b,d) so the
    per-sample GEMM lhsT slices stay contiguous.
  - All bulk data is bf16; y1 stays resident in SBUF.

kernel(**inputs) takes full unsharded inputs, shards/transposes on the host,
runs the SPMD kernel, reassembles the full [S,B,H] output.
"""

import numpy as np

import concourse.bass as bass  # noqa: F401
import concourse.mybir as mybir
from concourse import bacc
from concourse.tile import TileContext

H = 1024
DFF = 4096
E = 8
B = 8
S = 512
NC = 8
DSL = DFF // NC  # 512, per-core DFF slice
P = 128
F32 = mybir.dt.float32
BF16 = mybir.dt.bfloat16
AF = mybir.ActivationFunctionType


def build_module(time_loop=0, time_phase=0):
    """time_loop=R wraps the FFN phases (not gate/collectives) in an
    on-device For loop for timing runs; outputs are then meaningless."""
    nc = bacc.Bacc()

    # ---- I/O (all per-core) ----
    xt_all = nc.declare_dram_parameter("xt_all", [B, H, S], BF16, isOutput=False)
    xt_own = nc.declare_dram_parameter("xt_own", [H, S], BF16, isOutput=False)
    gw1t = nc.declare_dram_parameter("gw1t", [H, H], BF16, isOutput=False)
    gb1t = nc.declare_dram_parameter("gb1t", [P, 8], F32, isOutput=False)
    gw2t = nc.declare_dram_parameter("gw2t", [H, E], BF16, isOutput=False)
    gb2 = nc.declare_dram_parameter("gb2", [E], F32, isOutput=False)
    # ew1s[e + 8c, (k*32 + q)*128 + i] = eW1[e, j*512 + q*16 + c, k*128 + i]
    ew1s = nc.declare_dram_parameter("ew1s", [P, 8 * 32 * P], BF16, isOutput=False)
    # ew2s[e + 8c, ((g*4 + kt)*8 + hh)*128 + d]
    #   = eW2[e, (g*8 + hh)*16 + c, j*512 + kt*128 + d]   (hgg-major chunks)
    ew2s = nc.declare_dram_parameter("ew2s", [P, 4 * 64 * P], BF16, isOutput=False)
    # gmask[e + 8c, b*16 + c2] = (c == c2)
    gmask = nc.declare_dram_parameter("gmask", [P, P], BF16, isOutput=False)
    # maskb[b'*8+e', b*16+c2] = (b' == b);  maske[b'*8+e', c*8+e] = (e' == e)
    maskb = nc.declare_dram_parameter("maskb", [8 * E, P], BF16, isOutput=False)
    maske = nc.declare_dram_parameter("maske", [8 * E, P], BF16, isOutput=False)
    eb1s = nc.declare_dram_parameter("eb1s", [E, DSL], BF16, isOutput=False)
    eb2 = nc.declare_dram_parameter("eb2", [E, H], F32, isOutput=False)
    y_out = nc.declare_dram_parameter("y2t", [H, S], BF16, isOutput=True)

    # ---- internal DRAM ----
    ag_in = nc.dram_tensor("ag_in", [E], F32)
    ag_out = nc.dram_tensor("ag_out", [NC * E], F32, addr_space="Shared")
    rs_in = nc.dram_tensor("rs_in", [2, 2, B, 2, P, S], BF16)
    rs_out = nc.dram_tensor("rs_out", [2, 2, 2 * P, S], BF16)
    groups = [list(range(NC))]

    with TileContext(nc) as tc:
        with (
            tc.tile_pool(name="main", bufs=1) as pool,
            tc.tile_pool(name="psum", bufs=1, space="PSUM") as pp,
        ):
            # W: single self-aliasing weight buffer, slot layout
            # Wd[p, s, b, d] = column (s*8 + b)*512 + d.
            #   w2(kt,b,h) at slot kt*2 + h//512, off h%512 (canonical)
            #   w1'(k,b,d), k<6: slot ((k%2)*2 + (d//128)//2)*2 + k//4,
            #                    off ((k//2)%2)*256 + ((d//128)%2)*128 + d%128
            #   w1'(k=6,7): separate w1x tile
            Wt = pool.tile([P, 32768], BF16, tag="W")
            Wd = Wt[:].rearrange("p (s b d) -> p s b d", s=8, b=8)
            w1x = pool.tile([P, 2 * 8 * 512], BF16, tag="w1x")
            w1xd = w1x[:].rearrange("p (k b d) -> p k b d", k=2, b=8)

            # =================== gate (own sample) ===================
            xo = pool.tile([P, 8 * S], BF16, tag="x8", bufs=3)
            nc.sync.dma_start(
                xo[:].rearrange("p (k s) -> p k s", k=8),
                xt_own.rearrange("(k p) s -> p k s", p=P),
            )
            gb1_sb = pool.tile([P, 8], F32, tag="gb1")
            nc.sync.dma_start(gb1_sb[:], gb1t[:])
            gw1v = Wt[:, 0:16384].rearrange("p (k o) -> p k o", k=8)[:, :, :H]
            for k in range(8):
                nc.sync.dma_start(gw1v[:, k], gw1t[k * P : (k + 1) * P, :])
            h1 = pool.tile([P, 8 * S], BF16, tag="x8", bufs=3)
            h1v = h1[:].rearrange("p (m s) -> p m s", m=8)
            for m in range(8):
                ps = pp.tile([P, S], F32, tag="out", bufs=3)
                for k in range(8):
                    nc.tensor.matmul(
                        ps[:],
                        gw1v[:, k, m * P : (m + 1) * P],
                        xo[:, k * S : (k + 1) * S],
                        start=(k == 0),
                        stop=(k == 7),
                    )
                nc.scalar.activation(
                    h1v[:, m], ps[:], AF.Relu, bias=gb1_sb[:, m : m + 1]
                )
            gw2_sb = pool.tile([P, 8 * E], BF16, tag="gw2")
            for k in range(8):
                nc.sync.dma_start(
                    gw2_sb[:, k * E : (k + 1) * E], gw2t[k * P : (k + 1) * P, :]
                )
            ps_g = pp.tile([E, S], F32, tag="tiny")
            for k in range(8):
                nc.tensor.matmul(
                    ps_g[:],
                    gw2_sb[:, k * E : (k + 1) * E],
                    h1v[:, k],
                    start=(k == 0),
                    stop=(k == 7),
                )
            gsum = pool.tile([E, 1], F32, tag="gsum")
            nc.vector.reduce_sum(gsum[:], ps_g[:], axis=mybir.AxisListType.X)
            gb2_sb = pool.tile([E, 1], F32, tag="gb2")
            nc.gpsimd.dma_start(gb2_sb[:], gb2[:, None])
            gmean = pool.tile([E, 1], F32, tag="gmean")
            nc.vector.tensor_scalar_mul(gmean[:], gsum[:], 1.0 / S)
            gown = pool.tile([E, 1], F32, tag="gown")
            nc.vector.tensor_add(gown[:], gmean[:], gb2_sb[:])
            nc.sync.dma_start(ag_in[:], gown[:, 0])

            nc.gpsimd.collective_compute(
                "AllGather",
                mybir.AluOpType.bypass,
                ins=[ag_in[:]],
                outs=[ag_out[:]],
                replica_groups=groups,
            )

            # ---- block-diagonal g (moving operand of all merges) ----
            # g64[b*8+e, 0] = g[b, e]; rhsg = maskb * g64 (per-part scalar);
            # garr[(c e), (b c2)] = maske.T @ rhsg = g[b, e];
            # gblk = garr * gmask  (bf16)
            gmask_sb = pool.tile([P, P], BF16, tag="gmask")
            nc.sync.dma_start(gmask_sb[:], gmask[:])
            maskb_sb = pool.tile([8 * E, P], BF16, tag="maskb")
            nc.sync.dma_start(maskb_sb[:], maskb[:])
            maske_sb = pool.tile([8 * E, P], BF16, tag="maske")
            nc.sync.dma_start(maske_sb[:], maske[:])
            g64 = pool.tile([8 * E, 1], F32, tag="g64")
            nc.gpsimd.dma_start(g64[:], ag_out[:, None])
            rhsg = pool.tile([8 * E, P], BF16, tag="rhsg")
            nc.vector.tensor_scalar_mul(rhsg[:], maskb_sb[:], g64[:, 0:1])
            garr_ps = pp.tile([P, P], F32, tag="tiny")
            nc.tensor.matmul(garr_ps[:], maske_sb[:], rhsg[:], start=True, stop=True)
            gblk = pool.tile([P, P], BF16, tag="gblk")
            nc.vector.tensor_mul(gblk[:], garr_ps[:], gmask_sb[:])

            gT_f = pool.tile([E, B], F32, tag="gTf")
            nc.gpsimd.dma_start(gT_f[:], ag_out.rearrange("(b e) -> e b", e=E))
            gT_r = pool.tile([E, B], BF16, tag="gT")
            nc.vector.tensor_copy(gT_r[:], gT_f[:])

            # ---- merged per-sample biases ----
            # b1t[:, mt*8+b] = (g[b] @ eb1s)[mt-tile]      (full value)
            # b2t[:, m*8+b]  = (g[b] @ eb2)[m-tile] / 8    (1/8: summed by RS)
            eb1_r = pool.tile([E, DSL], BF16, tag="eb1")
            nc.gpsimd.dma_start(eb1_r[:], eb1s[:])
            eb2_f = pool.tile([E, H], F32, tag="eb2f")
            nc.gpsimd.dma_start(eb2_f[:], eb2[:])
            eb2_r8 = pool.tile([E, H], BF16, tag="eb2r")
            nc.scalar.activation(eb2_r8[:], eb2_f[:], AF.Copy, scale=1.0 / NC)
            b1t = pool.tile([P, 4 * B], F32, tag="b1t")
            b2t = pool.tile([P, 8 * B], F32, tag="b2t")
            for mt in range(4):
                ps = pp.tile([P, B], F32, tag="tiny")
                nc.tensor.matmul(
                    ps[:], eb1_r[:, mt * P : (mt + 1) * P], gT_r[:],
                    start=True, stop=True,
                )
                nc.vector.tensor_copy(b1t[:, mt * B : (mt + 1) * B], ps[:])
            for m in range(8):
                ps = pp.tile([P, B], F32, tag="tiny")
                nc.tensor.matmul(
                    ps[:], eb2_r8[:, m * P : (m + 1) * P], gT_r[:],
                    start=True, stop=True,
                )
                nc.vector.tensor_copy(b2t[:, m * B : (m + 1) * B], ps[:])

            # y1 stays in SBUF: [P(dff-part), b, kt, s]
            y1 = pool.tile([P, B * 4 * S], BF16, tag="y1")
            y1v = y1[:].rearrange("p (b t s) -> p b t s", b=B, t=4)

            # ---- merge drains alternate DVE / ACT ----
            def dve_copy(d, s):
                nc.vector.tensor_copy(d, s)

            def act_copy(d, s):
                nc.scalar.activation(d, s, AF.Copy)

            DRAIN = [dve_copy, act_copy]

            def load_e1(k):
                t = pool.tile([P, 4096], BF16, tag="e1", bufs=3)
                nc.sync.dma_start(t[:], ew1s[:, k * 4096 : (k + 1) * 4096])
                return t[:].rearrange("p (q i) -> p q i", q=32)

            def load_e2(g):
                t = pool.tile([P, 4096], BF16, tag="e2", bufs=3)
                nc.sync.dma_start(t[:], ew2s[:, g * 4096 : (g + 1) * 4096])
                return t[:].rearrange("p (kt hh i) -> p kt hh i", kt=4, hh=8)

            def load_xb():
                xb = pool.tile([P, 8 * S], BF16, tag="x8", bufs=3, name="xb")
                return xb

            def xb_dma(xb, b):
                nc.sync.dma_start(
                    xb[:].rearrange("p (k s) -> p k s", k=8),
                    xt_all.rearrange("b (k p) s -> b p k s", p=P)[b],
                )

            def w1_slot(k, w):
                # (slot, col offset) of w1' d-window w (128 wide) of k-tile k
                s = ((k % 2) * 2 + w // 2) * 2 + k // 4
                off = ((k // 2) % 2) * 256 + (w % 2) * 128
                return s, off

            def merge1_bank(e1v, k, qg, eng):
                mm = pp.tile([P, 1024], F32, tag="mm", bufs=2)
                for qq in range(8):
                    nc.tensor.matmul(
                        mm[:, qq * P : (qq + 1) * P],
                        e1v[:, qg * 8 + qq],
                        gblk[:],
                        start=True, stop=True,
                    )
                src = mm[:].rearrange("p (q b c) -> p b q c", q=8, b=8)
                if k >= 6:
                    dst = w1xd[:, k - 6, :, qg * 128 : (qg + 1) * 128]
                else:
                    s, off = w1_slot(k, qg)
                    dst = Wd[:, s, :, off : off + 128]
                DRAIN[eng](dst.rearrange("p b (q c) -> p b q c", q=8), src)

            def merge2_bank(e2v_g, g, kt, eng):
                mm = pp.tile([P, 1024], F32, tag="mm", bufs=2)
                for hh in range(8):
                    nc.tensor.matmul(
                        mm[:, hh * P : (hh + 1) * P],
                        e2v_g[:, kt, hh],
                        gblk[:],
                        start=True, stop=True,
                    )
                src = mm[:].rearrange("p (q b c) -> p b q c", q=8, b=8)
                dst = Wd[:, kt * 2 + g // 4, :, (g % 4) * P : (g % 4 + 1) * P]
                DRAIN[eng](dst.rearrange("p b (q c) -> p b q c", q=8), src)

            def gemm1_sample(b, xb):
                xbv = xb[:].rearrange("p (k s) -> p k s", k=8)
                for mt in range(4):
                    ps = pp.tile([P, S], F32, tag="out", bufs=3)
                    for k in range(8):
                        if k >= 6:
                            lhsT = w1xd[:, k - 6, b, mt * P : (mt + 1) * P]
                        else:
                            s, off = w1_slot(k, mt)
                            lhsT = Wd[:, s, b, off + 0 : off + P]
                        nc.tensor.matmul(
                            ps[:], lhsT, xbv[:, k],
                            start=(k == 0), stop=(k == 7),
                        )
                    nc.scalar.activation(
                        y1v[:, b, mt], ps[:], AF.Relu,
                        bias=b1t[:, mt * B + b : mt * B + b + 1],
                    )

            def merge1_full():
                e1v = {}
                for k in range(8):
                    e1v[k] = load_e1(k)
                di = 0
                for k in range(8):
                    for qg in range(4):
                        merge1_bank(e1v[k], k, qg, di % 2)
                        di += 1

            def body(emit_next, with_rs):
                e2v = {}
                xbs = [load_xb(), load_xb()]
                xb_dma(xbs[0], 0)
                xb_dma(xbs[1], 1)
                for b in range(B):
                    if b + 2 < B:
                        xbs.append(load_xb())
                        xb_dma(xbs[b + 2], b + 2)
                    if b in (3, 4, 6):
                        g = {3: 0, 4: 1, 6: 2}[b]
                        e2v[g] = load_e2(g)
                    gemm1_sample(b, xbs[b])
                # lead-in: merge2 banks needed by gemm2 chunk 0
                for i, g in enumerate((0, 1)):
                    for kt in range(4):
                        merge2_bank(e2v[g], g, kt, (i * 4 + kt) % 2)
                e1v = {}
                for cg in range(4):
                    for g in {0: (3, 4), 1: (5, 6), 2: (7,)}.get(cg, ()):
                        e2v[g] = load_e2(g)
                    if emit_next:
                        for k in {0: (0, 1), 1: (2, 3), 2: (4, 5, 6),
                                  3: (7,)}[cg]:
                            e1v[k] = load_e1(k)
                    banks = []
                    if cg < 3:
                        for g in (2 * cg + 2, 2 * cg + 3):
                            for kt in range(4):
                                banks.append(("m2", g, kt))
                    if emit_next and cg >= 1:
                        for k in (2 * (cg - 1), 2 * (cg - 1) + 1):
                            for qg in range(4):
                                banks.append(("m1", k, qg))
                    if emit_next and cg == 3:
                        for k in (6, 7):
                            for qg in range(4):
                                banks.append(("m1", k, qg))
                    nb = len(banks)
                    bi = 0
                    hh2, mp_ = cg // 2, cg % 2
                    for pi in range(16):
                        b, ml = pi // 2, pi % 2
                        mg = cg * 2 + ml
                        ps = pp.tile([P, S], F32, tag="out", bufs=3)
                        for kt in range(4):
                            nc.tensor.matmul(
                                ps[:],
                                Wd[:, kt * 2 + mg // 4, b,
                                   (mg % 4) * P : (mg % 4 + 1) * P],
                                y1v[:, b, kt],
                                start=(kt == 0), stop=(kt == 3),
                            )
                        y2 = pool.tile([P, S], BF16, tag="y2s", bufs=8)
                        nc.scalar.activation(
                            y2[:], ps[:], AF.Identity,
                            bias=b2t[:, mg * B + b : mg * B + b + 1],
                        )
                        nc.sync.dma_start(rs_in[hh2, mp_, b, ml], y2[:])
                        # interleave merge banks: drains mostly on DVE since
                        # ACT carries the y2 bias-drains
                        want = (pi + 1) * nb // 16
                        while bi < want:
                            kind, a1, a2 = banks[bi]
                            eng = 0 if bi % 3 < 2 else 1
                            if kind == "m2":
                                merge2_bank(e2v[a1], a1, a2, eng)
                            else:
                                merge1_bank(e1v[a1], a1, a2, eng)
                            bi += 1
                    if with_rs:
                        nc.gpsimd.collective_compute(
                            "ReduceScatter",
                            mybir.AluOpType.add,
                            ins=[
                                rs_in.ap()[hh2, mp_].rearrange(
                                    "b m p s -> (b m p) s"
                                )
                            ],
                            outs=[rs_out[hh2, mp_]],
                            replica_groups=groups,
                        )

            merge1_full()
            if time_loop:
                with tc.For_i(0, time_loop, 1):
                    body(emit_next=True, with_rs=False)
                nc.sync.dma_start(y_out[0 : 2 * P], rs_in.ap()[0, 0, 0])
            else:
                body(emit_next=False, with_rs=True)
                for hh2 in range(2):
                    for mp_ in range(2):
                        nc.sync.dma_start(
                            y_out[(hh2 * 4 + mp_ * 2) * P : (hh2 * 4 + mp_ * 2 + 2) * P],
                            rs_out[hh2, mp_],
                        )

    nc.compile()
    return nc


# ---------------- host-side sharding ----------------

def _bf16(a):
    import ml_dtypes
    return np.asarray(a, np.float32).astype(ml_dtypes.bfloat16)


def _ew1_dev(a):
    # a: [E, DSL(d), H(i)] -> [P, (k q i)] with partition (c e): p = 8c + e
    # ew1s[8c + e, (k*32 + q)*128 + i'] = a[e, q*16 + c, k*128 + i']
    t = np.asarray(a, np.float32).reshape(E, 32, 16, 8, P)  # [e, q, c, k, i']
    arr = t.transpose(2, 0, 3, 1, 4)  # [c, e, k, q, i']
    return _bf16(np.ascontiguousarray(arr.reshape(P, 8 * 32 * P)))


def _ew2_dev(c):
    # c: [E, H(h), DSL(d)] -> [P, (g kt hh d')] with partition (c e)
    # ew2s[8c + e, ((g*4 + kt)*8 + hh)*128 + d'] = c[e, (g*8+hh)*16 + c, kt*128 + d']
    t = np.asarray(c, np.float32).reshape(E, 8, 8, 16, 4, P)  # [e, g, hh, c, kt, d']
    arr = t.transpose(3, 0, 1, 4, 2, 5)  # [c, e, g, kt, hh, d']
    return _bf16(np.ascontiguousarray(arr.reshape(P, 4 * 64 * P)))


def _gmask():
    m = np.zeros((P, P), np.float32)
    for c in range(16):
        m[c * 8 : (c + 1) * 8, c::16] = 1.0
    return _bf16(m)


def _maskb():
    m = np.zeros((8 * E, P), np.float32)
    for b in range(8):
        m[b * 8 : (b + 1) * 8, b * 16 : (b + 1) * 16] = 1.0
    return _bf16(m)


def _maske():
    m = np.zeros((8 * E, P), np.float32)
    for e in range(E):
        m[e::8, e::8] = 1.0
    return _bf16(m)


def _shard_inputs(hidden_states, gW1, gb1, gW2, gb2, eW1, eb1, eW2, eb2):
    xt_all = _bf16(
        np.ascontiguousarray(
            np.asarray(hidden_states, dtype=np.float32).transpose(1, 2, 0)
        )
    )  # [B, H, S]
    gW1t = _bf16(np.ascontiguousarray(np.asarray(gW1, np.float32).T))
    gb1t = np.ascontiguousarray(np.asarray(gb1, np.float32).reshape(8, P).T)
    gW2t = _bf16(np.ascontiguousarray(np.asarray(gW2, np.float32).T))
    gb2 = np.ascontiguousarray(np.asarray(gb2, np.float32))
    eW1 = np.asarray(eW1, np.float32)
    eW2 = np.asarray(eW2, np.float32)
    eb1 = np.asarray(eb1, np.float32)
    eb2 = np.ascontiguousarray(np.asarray(eb2, np.float32))
    gmask = _gmask()
    maskb_h = _maskb()
    maske_h = _maske()
    in_maps = []
    for j in range(NC):
        sl = slice(j * DSL, (j + 1) * DSL)
        in_maps.append(
            {
                "xt_all": xt_all,
                "xt_own": np.ascontiguousarray(xt_all[j]),
                "gw1t": gW1t,
                "gb1t": gb1t,
                "gw2t": gW2t,
                "gb2": gb2,
                "ew1s": _ew1_dev(eW1[:, sl, :]),
                "ew2s": _ew2_dev(eW2[:, :, sl]),
                "gmask": gmask,
                "maskb": maskb_h,
                "maske": maske_h,
                "eb1s": _bf16(np.ascontiguousarray(eb1[:, sl])),
                "eb2": eb2,
            }
        )
    return in_maps


# ---------------- SPMD runner (persistent jit over axon PJRT) -----------

_CACHE = {}


def _build_runner(time_loop=0, time_phase=0):
    import jax
    from jax.sharding import Mesh, PartitionSpec
    from jax.experimental.shard_map import shard_map
    from concourse import bass2jax

    nc = build_module(time_loop=time_loop, time_phase=time_phase)
    bass2jax.install_neuronx_cc_hook()
    partition_name = nc.partition_id_tensor.name if nc.partition_id_tensor else None

    in_names, out_names, out_avals = [], [], []
    for alloc in nc.m.functions[0].allocations:
        if not isinstance(alloc, mybir.MemoryLocationSet):
            continue
        name = alloc.memorylocations[0].name
        if alloc.kind == "ExternalInput":
            if name != partition_name:
                in_names.append(name)
        elif alloc.kind == "ExternalOutput":
            out_avals.append(
                jax.core.ShapedArray(
                    tuple(alloc.tensor_shape), mybir.dt.np(alloc.dtype)
                )
            )
            out_names.append(name)
    n_outs = len(out_names)
    all_in_names = list(in_names) + list(out_names)
    if partition_name is not None:
        all_in_names.append(partition_name)

    def _body(*args):
        operands = list(args)
        if partition_name is not None:
            operands.append(bass2jax.partition_id_tensor())
        return tuple(
            bass2jax._bass_exec_p.bind(
                *operands,
                out_avals=tuple(out_avals),
                in_names=tuple(all_in_names),
                out_names=tuple(out_names),
                lowering_input_output_aliases=(),
                sim_require_finite=True,
                sim_require_nnan=True,
                nc=nc,
            )
        )

    devices = jax.devices()[:NC]
    mesh = Mesh(np.asarray(devices), ("core",))
    n_params = len(in_names)
    sharded = jax.jit(
        shard_map(
            _body,
            mesh=mesh,
            in_specs=(PartitionSpec("core"),) * (n_params + n_outs),
            out_specs=(PartitionSpec("core"),) * n_outs,
            check_rep=False,
        ),
        keep_unused=True,
    )
    zero_shapes = [((NC * a.shape[0], *a.shape[1:]), a.dtype) for a in out_avals]

    def run(in_maps, device_inputs=None, fetch=True):
        if device_inputs is None:
            concat_in = [
                np.concatenate(
                    [np.asarray(in_maps[c][n]) for c in range(NC)], axis=0
                )
                for n in in_names
            ]
            dev_params = [jax.device_put(x) for x in concat_in]
            dev_zeros = [jax.device_put(np.zeros(s, d)) for s, d in zero_shapes]
            device_inputs = (dev_params, dev_zeros)
            jax.block_until_ready(dev_params)
            jax.block_until_ready(dev_zeros)
        dev_params, dev_zeros = device_inputs
        out_arrs = sharded(*dev_params, *dev_zeros)
        jax.block_until_ready(out_arrs)
        if not fetch:
            return None, device_inputs
        results = [
            {
                name: np.asarray(out_arrs[i]).reshape(NC, *out_avals[i].shape)[c]
                for i, name in enumerate(out_names)
            }
            for c in range(NC)
        ]
        return results, device_inputs

    return run


def get_runner(time_loop=0, time_phase=0):
    key = ("run", time_loop, time_phase)
    if key not in _CACHE:
        _CACHE[key] = _build_runner(time_loop=time_loop, time_phase=time_phase)
    return _CACHE[key]


def kernel(**inputs) -> np.ndarray:
    run = get_runner()
    in_maps = _shard_inputs(**inputs)
    results, _ = run(in_maps)
    # core b's output is y2^T[b] = [H, S] bf16; assemble [S, B, H] f32
    y2t = np.stack(
        [results[b]["y2t"].astype(np.float32) for b in range(B)], axis=0
    )  # [B, H, S]
    return np.ascontiguousarray(y2t.transpose(2, 0, 1)).astype(np.float32)


def build_collective_bench(reps):
    """Standalone module issuing `reps` x (AllGather + 4 RS chunks),
    serialized by WAR on rs_out, for timing the collective stream."""
    nc = bacc.Bacc()
    xt_all = nc.declare_dram_parameter("xt_all", [B, H, S], BF16, isOutput=False)
    gb2 = nc.declare_dram_parameter("gb2", [E], F32, isOutput=False)
    y_out = nc.declare_dram_parameter("y2t", [H, S], BF16, isOutput=True)
    ag_in = nc.dram_tensor("ag_in", [E], F32)
    ag_out = nc.dram_tensor("ag_out", [NC * E], F32, addr_space="Shared")
    rs_in = nc.dram_tensor("rs_in", [2, 2, B, 2, P, S], BF16)
    rs_out = nc.dram_tensor("rs_out", [2, 2, 2 * P, S], BF16)
    groups = [list(range(NC))]
    with TileContext(nc) as tc:  # noqa: F841
        nc.sync.dma_start(
            rs_in.ap().rearrange("a c b m p s -> (a c b m p) s"),
            xt_all.ap().rearrange("b (r p) s -> (b r p) s", p=P),
        )
        nc.sync.dma_start(ag_in[:], gb2[:])
        for _ in range(reps):
            nc.gpsimd.collective_compute(
                "AllGather", mybir.AluOpType.bypass,
                ins=[ag_in[:]], outs=[ag_out[:]], replica_groups=groups,
            )
            for h in range(2):
                for mp in range(2):
                    nc.gpsimd.collective_compute(
                        "ReduceScatter", mybir.AluOpType.add,
                        ins=[rs_in.ap()[h, mp].rearrange("b m p s -> (b m p) s")],
                        outs=[rs_out[h, mp]], replica_groups=groups,
                    )
        nc.sync.dma_start(y_out[0 : 2 * P], rs_out[0, 0])
    nc.compile()
    return nc


def get_collective_runner(reps):
    key = ("coll", reps)
    if key not in _CACHE:
        import functools
        global build_module
        orig = build_module
        try:
            build_module = functools.partial(_cb_shim, reps)
            _CACHE[key] = _build_runner(time_loop=0, time_phase=0)
        finally:
            build_module = orig
    return _CACHE[key]


def _cb_shim(reps, time_loop=0, time_phase=0):
    return build_collective_bench(reps)
